# revision 18
# baseline (speedup 1.0000x reference)
import sys

import numpy as np

for p in ("/opt/trn_rl_repo",):
    if p not in sys.path:
        sys.path.insert(0, p)

import concourse.bass as bass  # noqa: E402
import concourse.tile as tile  # noqa: E402
from concourse import bacc, mybir  # noqa: E402
from concourse.bass_utils import run_bass_kernel_spmd  # noqa: E402

B, N, D = 128, 512, 512
NCORES = 8
BPC = B // NCORES  # 16 batch items per core
F32 = mybir.dt.float32
BF16 = mybir.dt.bfloat16


def _hadamard(n: int) -> np.ndarray:
    H = np.array([[1.0]], dtype=np.float32)
    base = np.array([[1.0, 1.0], [1.0, -1.0]], dtype=np.float32)
    while H.shape[0] < n:
        H = np.kron(H, base)
    return H


def _build():
    # Row m = 4q+j lives on partition q, free block j (4 consecutive DRAM
    # rows per partition -> one 512KB DMA per batch with 4KB lines).
    # H512[4q+j, 4p+i] = H128[q,p]*H4[j,i]   (H512 = H128 (x) H4)
    # H512[dc*128+r, f*128+s] = H4[dc,f]*H128[r,s]  (H512 = H4 (x) H128)
    # Both transforms fold one H2 level into PSUM accumulation via paired
    # moving operands [S|S] / [S|-S] (N=256 matmuls); the remaining H2
    # level is one add + one sub on [128,1024] tiles (DVE, 2x rate).
    # Output leaves in (half, j, f', s) column order; host unpermutes.
    nc = bacc.Bacc("TRN2", target_bir_lowering=False, debug=False)
    x_d = nc.dram_tensor("x", [BPC, 128, 4 * D], BF16, kind="ExternalInput").ap()
    s4_d = nc.dram_tensor("s4", [128, 512], BF16, kind="ExternalInput").ap()
    s4b_d = nc.dram_tensor("s4b", [128, 512], BF16, kind="ExternalInput").ap()
    y_d = nc.dram_tensor("y", [BPC, 128, 4 * D], BF16, kind="ExternalOutput").ap()

    with tile.TileContext(nc) as tc:
        with (
            tc.tile_pool(name="const", bufs=1) as const_pool,
            tc.tile_pool(name="xp", bufs=3) as x_pool,
            tc.tile_pool(name="cs", bufs=3) as cs_pool,
            tc.tile_pool(name="csb", bufs=3) as csb_pool,
            tc.tile_pool(name="tp", bufs=3) as t_pool,
            tc.tile_pool(name="yp", bufs=3) as y_pool,
            tc.tile_pool(name="psa", bufs=1, space="PSUM") as psum_a,
            tc.tile_pool(name="psb", bufs=1, space="PSUM") as psum_b,
        ):
            s4_sb = const_pool.tile([128, 512], BF16, tag="s4")
            s4b_sb = const_pool.tile([128, 512], BF16, tag="s4b")

            sp2 = s4_sb[:, 0:256]  # [S | S]
            sn2 = s4_sb[:, 256:512]  # [S | -S]
            sp2b = s4b_sb[:, 0:256]  # [S | S] / 512
            sn2b = s4b_sb[:, 256:512]  # [S | -S] / 512

            state = None  # pending (ttP, ttM, b) from previous batch

            def emit_B(ttP, ttM, bprev):
                # B: y[4p+j, f*128+s] = sum_dc H4[dc,f] v_dc[4p+j, s]
                # v_dc[n, s] = sum_r t_T[dc*128+r, n] * H128[r,s]/512
                psB01 = psum_b.tile([128, 1024], F32, tag="b01")
                psB23 = psum_b.tile([128, 1024], F32, tag="b23")
                for j in range(4):
                    o = slice(j * 256, j * 256 + 256)
                    src = ttP if j < 2 else ttM
                    jj = (j % 2) * 128
                    t0 = src[:, 0 * 256 + jj : 0 * 256 + jj + 128]
                    t1 = src[:, 1 * 256 + jj : 1 * 256 + jj + 128]
                    t2 = src[:, 2 * 256 + jj : 2 * 256 + jj + 128]
                    t3 = src[:, 3 * 256 + jj : 3 * 256 + jj + 128]
                    nc.tensor.matmul(psB01[:, o], t0, sp2b, start=True, stop=False)
                    nc.tensor.matmul(psB01[:, o], t1, sn2b, start=False, stop=True)
                    nc.tensor.matmul(psB23[:, o], t2, sp2b, start=True, stop=False)
                    nc.tensor.matmul(psB23[:, o], t3, sn2b, start=False, stop=True)
                csB01 = csb_pool.tile([128, 1024], BF16, tag="b01")
                nc.scalar.copy(csB01[:], psB01[:])
                yPM = y_pool.tile([128, 2048], BF16, tag="yPM")
                nc.vector.tensor_add(yPM[:, 0:1024], csB01[:], psB23[:])
                nc.vector.tensor_sub(yPM[:, 1024:2048], csB01[:], psB23[:])
                nc.sync.dma_start(y_d[bprev], yPM[:])

            for b in range(BPC):
                # x cols are dt-major: dt*512 + j*128 + dd
                xb = x_pool.tile([128, 4 * D], BF16)
                nc.sync.dma_start(xb[:, 0:1024], x_d[b][:, 0:1024])
                nc.sync.dma_start(xb[:, 1024:2048], x_d[b][:, 1024:2048])
                if b == 0:
                    nc.sync.dma_start(s4_sb[:], s4_d[:])
                    nc.sync.dma_start(s4b_sb[:], s4b_d[:])

                # A: u_j[dd, p] = sum_q x[4q+j, dt*128+dd] * H128[q, p]
                # psum free layout dt*256 + (s:0-127 | d:128-255)
                ps_sd01 = psum_a.tile([128, 1024], F32, tag="sd01")
                ps_sd23 = psum_a.tile([128, 1024], F32, tag="sd23")
                for dt in range(4):
                    o = slice(dt * 256, dt * 256 + 256)
                    x0 = xb[:, dt * 512 + 0 : dt * 512 + 128]
                    x1 = xb[:, dt * 512 + 128 : dt * 512 + 256]
                    x2 = xb[:, dt * 512 + 256 : dt * 512 + 384]
                    x3 = xb[:, dt * 512 + 384 : dt * 512 + 512]
                    nc.tensor.matmul(ps_sd01[:, o], x0, sp2, start=True, stop=False)
                    nc.tensor.matmul(ps_sd01[:, o], x1, sn2, start=False, stop=True)
                    nc.tensor.matmul(ps_sd23[:, o], x2, sp2, start=True, stop=False)
                    nc.tensor.matmul(ps_sd23[:, o], x3, sn2, start=False, stop=True)
                cs_sd01 = cs_pool.tile([128, 1024], BF16, tag="sd01")
                cs_sd23 = cs_pool.tile([128, 1024], BF16, tag="sd23")
                nc.scalar.copy(cs_sd01[:, 0:512], ps_sd01[:, 0:512])
                nc.scalar.copy(cs_sd01[:, 512:1024], ps_sd01[:, 512:1024])
                nc.scalar.copy(cs_sd23[:], ps_sd23[:])
                # Remaining H2 level: ttP = [t0|t1], ttM = [t2|t3]
                # layout dt*256 + i'*128 + p
                ttP = t_pool.tile([128, 1024], BF16, tag="P")
                ttM = t_pool.tile([128, 1024], BF16, tag="M")
                nc.vector.tensor_add(ttP[:], cs_sd01[:], cs_sd23[:])
                nc.vector.tensor_sub(ttM[:], cs_sd01[:], cs_sd23[:])
                if state is not None:
                    emit_B(*state)
                state = (ttP, ttM, b)

            emit_B(*state)

    nc.compile()
    return nc


_NC = None


def _get_nc():
    global _NC
    if _NC is None:
        _NC = _build()
    return _NC


def _in_maps(x: np.ndarray) -> list:
    import ml_dtypes

    bf16 = ml_dtypes.bfloat16
    x = np.asarray(x)
    xb = np.ascontiguousarray(x, dtype=np.float32).astype(bf16)
    H128 = _hadamard(128)
    s4 = np.ascontiguousarray(
        np.concatenate([H128, H128, H128, -H128], axis=1)
    ).astype(bf16)
    s4b = np.ascontiguousarray(s4.astype(np.float32) / np.float32(512.0)).astype(
        bf16
    )
    # device x cols are dt-major: dt*512 + j*128 + dd
    xb = (
        xb.reshape(B, 128, 4, 4, 128)  # (b, q, j, dt, dd)
        .transpose(0, 1, 3, 2, 4)  # (b, q, dt, j, dd)
        .reshape(B, 128, 4 * D)
    )
    return [
        {
            "x": np.ascontiguousarray(xb[i * BPC : (i + 1) * BPC]),
            "s4": s4,
            "s4b": s4b,
        }
        for i in range(NCORES)
    ]


def kernel(x: np.ndarray) -> np.ndarray:
    nc = _get_nc()
    res = run_bass_kernel_spmd(nc, _in_maps(x), list(range(NCORES))).results
    outs = []
    for r in res:
        # y cols: half(P/M)*1024 + j*256 + f2*128 + s ; rows: 4p+j on
        # partition p -> y[4p+j, (half*2+f2)*128+s]
        o = np.asarray(r["y"]).reshape(BPC, 128, 2, 4, 2, 128)
        o = o.transpose(0, 1, 3, 2, 4, 5).reshape(BPC, N, D)
        outs.append(o.astype(np.float32))
    return np.concatenate(outs, axis=0)


# revision 19
# speedup vs baseline: 1.0798x; 1.0798x over previous
import sys

import numpy as np

for p in ("/opt/trn_rl_repo",):
    if p not in sys.path:
        sys.path.insert(0, p)

import concourse.bass as bass  # noqa: E402
import concourse.tile as tile  # noqa: E402
from concourse import bacc, mybir  # noqa: E402
from concourse.bass_utils import run_bass_kernel_spmd  # noqa: E402

B, N, D = 128, 512, 512
NCORES = 8
BPC = B // NCORES  # 16 batch items per core
F32 = mybir.dt.float32
BF16 = mybir.dt.bfloat16


def _hadamard(n: int) -> np.ndarray:
    H = np.array([[1.0]], dtype=np.float32)
    base = np.array([[1.0, 1.0], [1.0, -1.0]], dtype=np.float32)
    while H.shape[0] < n:
        H = np.kron(H, base)
    return H


def _build():
    # Row m = 4q+j lives on partition q, free block j (4 consecutive DRAM
    # rows per partition -> one 512KB DMA per batch with 4KB lines).
    # H512[4q+j, 4p+i] = H128[q,p]*H4[j,i]   (H512 = H128 (x) H4)
    # H512[dc*128+r, f*128+s] = H4[dc,f]*H128[r,s]  (H512 = H4 (x) H128)
    # Both transforms fold one H2 level into PSUM accumulation via paired
    # moving operands [S|S] / [S|-S] (N=256 matmuls); the remaining H2
    # level is one add + one sub on [128,1024] tiles (DVE, 2x rate).
    # Output leaves in (half, j, f', s) column order; host unpermutes.
    nc = bacc.Bacc("TRN2", target_bir_lowering=False, debug=False)
    x_d = nc.dram_tensor("x", [BPC, 128, 4 * D], BF16, kind="ExternalInput").ap()
    s4_d = nc.dram_tensor("s4", [128, 512], BF16, kind="ExternalInput").ap()
    s4b_d = nc.dram_tensor("s4b", [128, 512], BF16, kind="ExternalInput").ap()
    y_d = nc.dram_tensor("y", [BPC, 128, 4 * D], BF16, kind="ExternalOutput").ap()

    with tile.TileContext(nc) as tc:
        with (
            tc.tile_pool(name="const", bufs=1) as const_pool,
            tc.tile_pool(name="xp", bufs=3) as x_pool,
            tc.tile_pool(name="cs", bufs=3) as cs_pool,
            tc.tile_pool(name="csb", bufs=3) as csb_pool,
            tc.tile_pool(name="tp", bufs=3) as t_pool,
            tc.tile_pool(name="yp", bufs=3) as y_pool,
            tc.tile_pool(name="psa", bufs=1, space="PSUM") as psum_a,
            tc.tile_pool(name="psb", bufs=1, space="PSUM") as psum_b,
        ):
            s4_sb = const_pool.tile([128, 512], BF16, tag="s4")
            s4b_sb = const_pool.tile([128, 512], BF16, tag="s4b")

            sp2 = s4_sb[:, 0:256]  # [S | S]
            sn2 = s4_sb[:, 256:512]  # [S | -S]
            sp2b = s4b_sb[:, 0:256]  # [S | S] / 512
            sn2b = s4b_sb[:, 256:512]  # [S | -S] / 512

            state = None  # pending (ttP, ttM, b) from previous batch

            def emit_B(ttP, ttM, bprev):
                # B: y[4p+j, f*128+s] = sum_dc H4[dc,f] v_dc[4p+j, s]
                # v_dc[n, s] = sum_r t_T[dc*128+r, n] * H128[r,s]/512
                psB01 = psum_b.tile([128, 1024], F32, tag="b01")
                psB23 = psum_b.tile([128, 1024], F32, tag="b23")
                for j in range(4):
                    o = slice(j * 256, j * 256 + 256)
                    src = ttP if j < 2 else ttM
                    jj = (j % 2) * 128
                    t0 = src[:, 0 * 256 + jj : 0 * 256 + jj + 128]
                    t1 = src[:, 1 * 256 + jj : 1 * 256 + jj + 128]
                    t2 = src[:, 2 * 256 + jj : 2 * 256 + jj + 128]
                    t3 = src[:, 3 * 256 + jj : 3 * 256 + jj + 128]
                    nc.tensor.matmul(psB01[:, o], t0, sp2b, start=True, stop=False)
                    nc.tensor.matmul(psB01[:, o], t1, sn2b, start=False, stop=True)
                    nc.tensor.matmul(psB23[:, o], t2, sp2b, start=True, stop=False)
                    nc.tensor.matmul(psB23[:, o], t3, sn2b, start=False, stop=True)
                csB01 = csb_pool.tile([128, 1024], BF16, tag="b01")
                csB23 = csb_pool.tile([128, 1024], BF16, tag="b23")
                nc.scalar.copy(csB01[:], psB01[:])
                nc.vector.tensor_copy(csB23[:], psB23[:])
                yPM = y_pool.tile([128, 2048], BF16, tag="yPM")
                nc.vector.tensor_add(yPM[:, 0:1024], csB01[:], csB23[:])
                nc.vector.tensor_sub(yPM[:, 1024:2048], csB01[:], csB23[:])
                nc.sync.dma_start(y_d[bprev], yPM[:])

            for b in range(BPC):
                # x cols are dt-major: dt*512 + j*128 + dd
                xb = x_pool.tile([128, 4 * D], BF16)
                nc.sync.dma_start(xb[:, 0:1024], x_d[b][:, 0:1024])
                nc.sync.dma_start(xb[:, 1024:2048], x_d[b][:, 1024:2048])
                if b == 0:
                    nc.sync.dma_start(s4_sb[:], s4_d[:])
                    nc.sync.dma_start(s4b_sb[:], s4b_d[:])

                # A: u_j[dd, p] = sum_q x[4q+j, dt*128+dd] * H128[q, p]
                # psum free layout dt*256 + (s:0-127 | d:128-255)
                ps_sd01 = psum_a.tile([128, 1024], F32, tag="sd01")
                ps_sd23 = psum_a.tile([128, 1024], F32, tag="sd23")
                for dt in range(4):
                    o = slice(dt * 256, dt * 256 + 256)
                    x0 = xb[:, dt * 512 + 0 : dt * 512 + 128]
                    x1 = xb[:, dt * 512 + 128 : dt * 512 + 256]
                    x2 = xb[:, dt * 512 + 256 : dt * 512 + 384]
                    x3 = xb[:, dt * 512 + 384 : dt * 512 + 512]
                    nc.tensor.matmul(ps_sd01[:, o], x0, sp2, start=True, stop=False)
                    nc.tensor.matmul(ps_sd01[:, o], x1, sn2, start=False, stop=True)
                    nc.tensor.matmul(ps_sd23[:, o], x2, sp2, start=True, stop=False)
                    nc.tensor.matmul(ps_sd23[:, o], x3, sn2, start=False, stop=True)
                cs_sd01 = cs_pool.tile([128, 1024], BF16, tag="sd01")
                cs_sd23 = cs_pool.tile([128, 1024], BF16, tag="sd23")
                nc.scalar.copy(cs_sd01[:, 0:512], ps_sd01[:, 0:512])
                nc.scalar.copy(cs_sd01[:, 512:1024], ps_sd01[:, 512:1024])
                nc.scalar.copy(cs_sd23[:], ps_sd23[:])
                # Remaining H2 level: ttP = [t0|t1], ttM = [t2|t3]
                # layout dt*256 + i'*128 + p
                ttP = t_pool.tile([128, 1024], BF16, tag="P")
                ttM = t_pool.tile([128, 1024], BF16, tag="M")
                nc.vector.tensor_add(ttP[:], cs_sd01[:], cs_sd23[:])
                nc.vector.tensor_sub(ttM[:], cs_sd01[:], cs_sd23[:])
                if state is not None:
                    emit_B(*state)
                state = (ttP, ttM, b)

            emit_B(*state)

    nc.compile()
    return nc


_NC = None


def _get_nc():
    global _NC
    if _NC is None:
        _NC = _build()
    return _NC


def _in_maps(x: np.ndarray) -> list:
    import ml_dtypes

    bf16 = ml_dtypes.bfloat16
    x = np.asarray(x)
    xb = np.ascontiguousarray(x, dtype=np.float32).astype(bf16)
    H128 = _hadamard(128)
    s4 = np.ascontiguousarray(
        np.concatenate([H128, H128, H128, -H128], axis=1)
    ).astype(bf16)
    s4b = np.ascontiguousarray(s4.astype(np.float32) / np.float32(512.0)).astype(
        bf16
    )
    # device x cols are dt-major: dt*512 + j*128 + dd
    xb = (
        xb.reshape(B, 128, 4, 4, 128)  # (b, q, j, dt, dd)
        .transpose(0, 1, 3, 2, 4)  # (b, q, dt, j, dd)
        .reshape(B, 128, 4 * D)
    )
    return [
        {
            "x": np.ascontiguousarray(xb[i * BPC : (i + 1) * BPC]),
            "s4": s4,
            "s4b": s4b,
        }
        for i in range(NCORES)
    ]


def kernel(x: np.ndarray) -> np.ndarray:
    nc = _get_nc()
    res = run_bass_kernel_spmd(nc, _in_maps(x), list(range(NCORES))).results
    outs = []
    for r in res:
        # y cols: half(P/M)*1024 + j*256 + f2*128 + s ; rows: 4p+j on
        # partition p -> y[4p+j, (half*2+f2)*128+s]
        o = np.asarray(r["y"]).reshape(BPC, 128, 2, 4, 2, 128)
        o = o.transpose(0, 1, 3, 2, 4, 5).reshape(BPC, N, D)
        outs.append(o.astype(np.float32))
    return np.concatenate(outs, axis=0)


# revision 24
# speedup vs baseline: 1.0873x; 1.0070x over previous
import sys

import numpy as np

for p in ("/opt/trn_rl_repo",):
    if p not in sys.path:
        sys.path.insert(0, p)

import concourse.bass as bass  # noqa: E402
import concourse.tile as tile  # noqa: E402
from concourse import bacc, mybir  # noqa: E402
from concourse.bass_utils import run_bass_kernel_spmd  # noqa: E402

B, N, D = 128, 512, 512
NCORES = 8
BPC = B // NCORES  # 16 batch items per core
F32 = mybir.dt.float32
BF16 = mybir.dt.bfloat16


def _hadamard(n: int) -> np.ndarray:
    H = np.array([[1.0]], dtype=np.float32)
    base = np.array([[1.0, 1.0], [1.0, -1.0]], dtype=np.float32)
    while H.shape[0] < n:
        H = np.kron(H, base)
    return H


def _build():
    # Row m = 4q+j lives on partition q, free block j (4 consecutive DRAM
    # rows per partition -> one 512KB DMA per batch with 4KB lines).
    # H512[4q+j, 4p+i] = H128[q,p]*H4[j,i]   (H512 = H128 (x) H4)
    # H512[dc*128+r, f*128+s] = H4[dc,f]*H128[r,s]  (H512 = H4 (x) H128)
    # Both transforms fold one H2 level into PSUM accumulation via paired
    # moving operands [S|S] / [S|-S] (N=256 matmuls); the remaining H2
    # level is one add + one sub on [128,1024] tiles (DVE, 2x rate).
    # Output leaves in (half, j, f', s) column order; host unpermutes.
    nc = bacc.Bacc("TRN2", target_bir_lowering=False, debug=False)
    x_d = nc.dram_tensor("x", [BPC, 128, 4 * D], BF16, kind="ExternalInput").ap()
    s4_d = nc.dram_tensor("s4", [128, 512], BF16, kind="ExternalInput").ap()
    s4b_d = nc.dram_tensor("s4b", [128, 512], BF16, kind="ExternalInput").ap()
    y_d = nc.dram_tensor("y", [BPC, 128, 4 * D], BF16, kind="ExternalOutput").ap()

    with tile.TileContext(nc) as tc:
        with (
            tc.tile_pool(name="const", bufs=1) as const_pool,
            tc.tile_pool(name="xp", bufs=3) as x_pool,
            tc.tile_pool(name="cs", bufs=2) as cs_pool,
            tc.tile_pool(name="csb", bufs=2) as csb_pool,
            tc.tile_pool(name="tp", bufs=2) as t_pool,
            tc.tile_pool(name="yp", bufs=2) as y_pool,
            tc.tile_pool(name="psa", bufs=1, space="PSUM") as psum_a,
            tc.tile_pool(name="psb", bufs=1, space="PSUM") as psum_b,
        ):
            s4_sb = const_pool.tile([128, 512], BF16, tag="s4")
            s4b_sb = const_pool.tile([128, 512], BF16, tag="s4b")

            sp2 = s4_sb[:, 0:256]  # [S | S]
            sn2 = s4_sb[:, 256:512]  # [S | -S]
            sp2b = s4b_sb[:, 0:256]  # [S | S] / 512
            sn2b = s4b_sb[:, 256:512]  # [S | -S] / 512

            state = None  # pending (ttP, ttM, b) from previous batch

            def emit_B(ttP, ttM, bprev, last=False):
                # B: y[4p+j, f*128+s] = sum_dc H4[dc,f] v_dc[4p+j, s]
                # v_dc[n, s] = sum_r t_T[dc*128+r, n] * H128[r,s]/512
                psB01 = psum_b.tile([128, 1024], F32, tag="b01")
                psB23 = psum_b.tile([128, 1024], F32, tag="b23")
                for j in range(4):
                    o = slice(j * 256, j * 256 + 256)
                    src = ttP if j < 2 else ttM
                    jj = (j % 2) * 128
                    t0 = src[:, 0 * 256 + jj : 0 * 256 + jj + 128]
                    t1 = src[:, 1 * 256 + jj : 1 * 256 + jj + 128]
                    t2 = src[:, 2 * 256 + jj : 2 * 256 + jj + 128]
                    t3 = src[:, 3 * 256 + jj : 3 * 256 + jj + 128]
                    nc.tensor.matmul(psB01[:, o], t0, sp2b, start=True, stop=False)
                    nc.tensor.matmul(psB01[:, o], t1, sn2b, start=False, stop=True)
                    nc.tensor.matmul(psB23[:, o], t2, sp2b, start=True, stop=False)
                    nc.tensor.matmul(psB23[:, o], t3, sn2b, start=False, stop=True)
                csB01 = csb_pool.tile([128, 1024], BF16, tag="b01")
                csB23 = csb_pool.tile([128, 1024], BF16, tag="b23")
                nc.scalar.copy(csB01[:], psB01[:])
                nc.vector.tensor_copy(csB23[:], psB23[:])
                yPM = y_pool.tile([128, 2048], BF16, tag="yPM")
                nc.vector.tensor_add(yPM[:, 0:1024], csB01[:], csB23[:])
                if last:
                    # shorten the final-batch tail: ship the P half while
                    # the M half is still being computed
                    nc.sync.dma_start(y_d[bprev][:, 0:1024], yPM[:, 0:1024])
                    nc.vector.tensor_sub(yPM[:, 1024:2048], csB01[:], csB23[:])
                    nc.sync.dma_start(y_d[bprev][:, 1024:2048], yPM[:, 1024:2048])
                else:
                    nc.vector.tensor_sub(yPM[:, 1024:2048], csB01[:], csB23[:])
                    nc.sync.dma_start(y_d[bprev], yPM[:])

            for b in range(BPC):
                # x cols are dt-major: dt*512 + j*128 + dd
                xb = x_pool.tile([128, 4 * D], BF16)
                if b == 0:
                    # consts first (tiny), then x0 in dt-chunks so A(0)
                    # can start right after chunk 0 lands
                    nc.sync.dma_start(s4_sb[:], s4_d[:])
                    nc.sync.dma_start(s4b_sb[:], s4b_d[:])
                    for c in range(4):
                        nc.sync.dma_start(
                            xb[:, c * 512 : (c + 1) * 512],
                            x_d[b][:, c * 512 : (c + 1) * 512],
                        )
                else:
                    nc.sync.dma_start(xb[:], x_d[b])

                # A: u_j[dd, p] = sum_q x[4q+j, dt*128+dd] * H128[q, p]
                # psum free layout dt*256 + (s:0-127 | d:128-255)
                ps_sd01 = psum_a.tile([128, 1024], F32, tag="sd01")
                ps_sd23 = psum_a.tile([128, 1024], F32, tag="sd23")
                for dt in range(4):
                    o = slice(dt * 256, dt * 256 + 256)
                    x0 = xb[:, dt * 512 + 0 : dt * 512 + 128]
                    x1 = xb[:, dt * 512 + 128 : dt * 512 + 256]
                    x2 = xb[:, dt * 512 + 256 : dt * 512 + 384]
                    x3 = xb[:, dt * 512 + 384 : dt * 512 + 512]
                    nc.tensor.matmul(ps_sd01[:, o], x0, sp2, start=True, stop=False)
                    nc.tensor.matmul(ps_sd01[:, o], x1, sn2, start=False, stop=True)
                    nc.tensor.matmul(ps_sd23[:, o], x2, sp2, start=True, stop=False)
                    nc.tensor.matmul(ps_sd23[:, o], x3, sn2, start=False, stop=True)
                cs_sd01 = cs_pool.tile([128, 1024], BF16, tag="sd01")
                cs_sd23 = cs_pool.tile([128, 1024], BF16, tag="sd23")
                nc.scalar.copy(cs_sd01[:, 0:512], ps_sd01[:, 0:512])
                nc.scalar.copy(cs_sd01[:, 512:1024], ps_sd01[:, 512:1024])
                nc.scalar.copy(cs_sd23[:], ps_sd23[:])
                # Remaining H2 level: ttP = [t0|t1], ttM = [t2|t3]
                # layout dt*256 + i'*128 + p
                ttP = t_pool.tile([128, 1024], BF16, tag="P")
                ttM = t_pool.tile([128, 1024], BF16, tag="M")
                nc.vector.tensor_add(ttP[:], cs_sd01[:], cs_sd23[:])
                nc.vector.tensor_sub(ttM[:], cs_sd01[:], cs_sd23[:])
                if state is not None:
                    emit_B(*state)
                state = (ttP, ttM, b)

            emit_B(*state, last=True)

    nc.compile()
    return nc


_NC = None


def _get_nc():
    global _NC
    if _NC is None:
        _NC = _build()
    return _NC


def _in_maps(x: np.ndarray) -> list:
    import ml_dtypes

    bf16 = ml_dtypes.bfloat16
    x = np.asarray(x)
    xb = np.ascontiguousarray(x, dtype=np.float32).astype(bf16)
    H128 = _hadamard(128)
    s4 = np.ascontiguousarray(
        np.concatenate([H128, H128, H128, -H128], axis=1)
    ).astype(bf16)
    s4b = np.ascontiguousarray(s4.astype(np.float32) / np.float32(512.0)).astype(
        bf16
    )
    # device x cols are dt-major: dt*512 + j*128 + dd
    xb = (
        xb.reshape(B, 128, 4, 4, 128)  # (b, q, j, dt, dd)
        .transpose(0, 1, 3, 2, 4)  # (b, q, dt, j, dd)
        .reshape(B, 128, 4 * D)
    )
    return [
        {
            "x": np.ascontiguousarray(xb[i * BPC : (i + 1) * BPC]),
            "s4": s4,
            "s4b": s4b,
        }
        for i in range(NCORES)
    ]


def kernel(x: np.ndarray) -> np.ndarray:
    nc = _get_nc()
    res = run_bass_kernel_spmd(nc, _in_maps(x), list(range(NCORES))).results
    outs = []
    for r in res:
        # y cols: half(P/M)*1024 + j*256 + f2*128 + s ; rows: 4p+j on
        # partition p -> y[4p+j, (half*2+f2)*128+s]
        o = np.asarray(r["y"]).reshape(BPC, 128, 2, 4, 2, 128)
        o = o.transpose(0, 1, 3, 2, 4, 5).reshape(BPC, N, D)
        outs.append(o.astype(np.float32))
    return np.concatenate(outs, axis=0)


# revision 25
# speedup vs baseline: 1.2933x; 1.1894x over previous
import sys

import numpy as np

for p in ("/opt/trn_rl_repo",):
    if p not in sys.path:
        sys.path.insert(0, p)

import concourse.bass as bass  # noqa: E402
import concourse.tile as tile  # noqa: E402
from concourse import bacc, mybir  # noqa: E402
from concourse.bass_utils import run_bass_kernel_spmd  # noqa: E402

B, N, D = 128, 512, 512
NCORES = 8
BPC = B // NCORES  # 16 batch items per core
F32 = mybir.dt.float32
BF16 = mybir.dt.bfloat16


def _hadamard(n: int) -> np.ndarray:
    H = np.array([[1.0]], dtype=np.float32)
    base = np.array([[1.0, 1.0], [1.0, -1.0]], dtype=np.float32)
    while H.shape[0] < n:
        H = np.kron(H, base)
    return H


def _build():
    # Row m = 4q+j lives on partition q, free block j (4 consecutive DRAM
    # rows per partition -> one 512KB DMA per batch with 4KB lines).
    # H512[4q+j, 4p+i] = H128[q,p]*H4[j,i]   (H512 = H128 (x) H4)
    # H512[dc*128+r, f*128+s] = H4[dc,f]*H128[r,s]  (H512 = H4 (x) H128)
    # Both transforms fold one H2 level into PSUM accumulation via paired
    # moving operands [S|S] / [S|-S] (N=256 matmuls); the remaining H2
    # level is one add + one sub on [128,1024] tiles (DVE, 2x rate).
    # Output leaves in (half, j, f', s) column order; host unpermutes.
    nc = bacc.Bacc("TRN2", target_bir_lowering=False, debug=False)
    x_d = nc.dram_tensor("x", [BPC, 128, 4 * D], BF16, kind="ExternalInput").ap()
    s4_d = nc.dram_tensor("s4", [128, 512], BF16, kind="ExternalInput").ap()
    s4b_d = nc.dram_tensor("s4b", [128, 512], BF16, kind="ExternalInput").ap()
    y_d = nc.dram_tensor("y", [BPC, 128, 4 * D], BF16, kind="ExternalOutput").ap()

    with tile.TileContext(nc) as tc:
        with (
            tc.tile_pool(name="const", bufs=1) as const_pool,
            tc.tile_pool(name="xp", bufs=3) as x_pool,
            tc.tile_pool(name="cs", bufs=3) as cs_pool,
            tc.tile_pool(name="csb", bufs=3) as csb_pool,
            tc.tile_pool(name="tp", bufs=3) as t_pool,
            tc.tile_pool(name="yp", bufs=3) as y_pool,
            tc.tile_pool(name="psa", bufs=1, space="PSUM") as psum_a,
            tc.tile_pool(name="psb", bufs=1, space="PSUM") as psum_b,
        ):
            s4_sb = const_pool.tile([128, 512], BF16, tag="s4")
            s4b_sb = const_pool.tile([128, 512], BF16, tag="s4b")

            sp2 = s4_sb[:, 0:256]  # [S | S]
            sn2 = s4_sb[:, 256:512]  # [S | -S]
            sp2b = s4b_sb[:, 0:256]  # [S | S] / 512
            sn2b = s4b_sb[:, 256:512]  # [S | -S] / 512

            state = None  # pending (ttP, ttM, b) from previous batch

            def emit_B(ttP, ttM, bprev, last=False):
                # B: y[4p+j, f*128+s] = sum_dc H4[dc,f] v_dc[4p+j, s]
                # v_dc[n, s] = sum_r t_T[dc*128+r, n] * H128[r,s]/512
                psB01 = psum_b.tile([128, 1024], F32, tag="b01")
                psB23 = psum_b.tile([128, 1024], F32, tag="b23")
                for j in range(4):
                    o = slice(j * 256, j * 256 + 256)
                    src = ttP if j < 2 else ttM
                    jj = (j % 2) * 128
                    t0 = src[:, 0 * 256 + jj : 0 * 256 + jj + 128]
                    t1 = src[:, 1 * 256 + jj : 1 * 256 + jj + 128]
                    t2 = src[:, 2 * 256 + jj : 2 * 256 + jj + 128]
                    t3 = src[:, 3 * 256 + jj : 3 * 256 + jj + 128]
                    nc.tensor.matmul(psB01[:, o], t0, sp2b, start=True, stop=False)
                    nc.tensor.matmul(psB01[:, o], t1, sn2b, start=False, stop=True)
                    nc.tensor.matmul(psB23[:, o], t2, sp2b, start=True, stop=False)
                    nc.tensor.matmul(psB23[:, o], t3, sn2b, start=False, stop=True)
                csB01 = csb_pool.tile([128, 1024], BF16, tag="b01")
                csB23 = csb_pool.tile([128, 1024], BF16, tag="b23")
                nc.scalar.copy(csB01[:], psB01[:])
                nc.vector.tensor_copy(csB23[:], psB23[:])
                yPM = y_pool.tile([128, 2048], BF16, tag="yPM")
                nc.vector.tensor_add(yPM[:, 0:1024], csB01[:], csB23[:])
                if last:
                    # shorten the final-batch tail: ship the P half while
                    # the M half is still being computed
                    nc.sync.dma_start(y_d[bprev][:, 0:1024], yPM[:, 0:1024])
                    nc.vector.tensor_sub(yPM[:, 1024:2048], csB01[:], csB23[:])
                    nc.sync.dma_start(y_d[bprev][:, 1024:2048], yPM[:, 1024:2048])
                else:
                    nc.vector.tensor_sub(yPM[:, 1024:2048], csB01[:], csB23[:])
                    nc.sync.dma_start(y_d[bprev], yPM[:])

            for b in range(BPC):
                # x cols are dt-major: dt*512 + j*128 + dd
                xb = x_pool.tile([128, 4 * D], BF16)
                if b == 0:
                    # consts first (tiny), then x0 in dt-chunks so A(0)
                    # can start right after chunk 0 lands
                    nc.sync.dma_start(s4_sb[:], s4_d[:])
                    nc.sync.dma_start(s4b_sb[:], s4b_d[:])
                    for c in range(4):
                        nc.sync.dma_start(
                            xb[:, c * 512 : (c + 1) * 512],
                            x_d[b][:, c * 512 : (c + 1) * 512],
                        )
                else:
                    nc.sync.dma_start(xb[:], x_d[b])

                # A: u_j[dd, p] = sum_q x[4q+j, dt*128+dd] * H128[q, p]
                # psum free layout dt*256 + (s:0-127 | d:128-255)
                ps_sd01 = psum_a.tile([128, 1024], F32, tag="sd01")
                ps_sd23 = psum_a.tile([128, 1024], F32, tag="sd23")
                for dt in range(4):
                    o = slice(dt * 256, dt * 256 + 256)
                    x0 = xb[:, dt * 512 + 0 : dt * 512 + 128]
                    x1 = xb[:, dt * 512 + 128 : dt * 512 + 256]
                    x2 = xb[:, dt * 512 + 256 : dt * 512 + 384]
                    x3 = xb[:, dt * 512 + 384 : dt * 512 + 512]
                    nc.tensor.matmul(ps_sd01[:, o], x0, sp2, start=True, stop=False)
                    nc.tensor.matmul(ps_sd01[:, o], x1, sn2, start=False, stop=True)
                    nc.tensor.matmul(ps_sd23[:, o], x2, sp2, start=True, stop=False)
                    nc.tensor.matmul(ps_sd23[:, o], x3, sn2, start=False, stop=True)
                cs_sd01 = cs_pool.tile([128, 1024], BF16, tag="sd01")
                cs_sd23 = cs_pool.tile([128, 1024], BF16, tag="sd23")
                nc.scalar.copy(cs_sd01[:, 0:512], ps_sd01[:, 0:512])
                nc.scalar.copy(cs_sd01[:, 512:1024], ps_sd01[:, 512:1024])
                nc.scalar.copy(cs_sd23[:], ps_sd23[:])
                # Remaining H2 level: ttP = [t0|t1], ttM = [t2|t3]
                # layout dt*256 + i'*128 + p
                ttP = t_pool.tile([128, 1024], BF16, tag="P")
                ttM = t_pool.tile([128, 1024], BF16, tag="M")
                nc.vector.tensor_add(ttP[:], cs_sd01[:], cs_sd23[:])
                nc.vector.tensor_sub(ttM[:], cs_sd01[:], cs_sd23[:])
                if state is not None:
                    emit_B(*state)
                state = (ttP, ttM, b)

            emit_B(*state, last=True)

    nc.compile()
    return nc


_NC = None


def _get_nc():
    global _NC
    if _NC is None:
        _NC = _build()
    return _NC


def _in_maps(x: np.ndarray) -> list:
    import ml_dtypes

    bf16 = ml_dtypes.bfloat16
    x = np.asarray(x)
    xb = np.ascontiguousarray(x, dtype=np.float32).astype(bf16)
    H128 = _hadamard(128)
    s4 = np.ascontiguousarray(
        np.concatenate([H128, H128, H128, -H128], axis=1)
    ).astype(bf16)
    s4b = np.ascontiguousarray(s4.astype(np.float32) / np.float32(512.0)).astype(
        bf16
    )
    # device x cols are dt-major: dt*512 + j*128 + dd
    xb = (
        xb.reshape(B, 128, 4, 4, 128)  # (b, q, j, dt, dd)
        .transpose(0, 1, 3, 2, 4)  # (b, q, dt, j, dd)
        .reshape(B, 128, 4 * D)
    )
    return [
        {
            "x": np.ascontiguousarray(xb[i * BPC : (i + 1) * BPC]),
            "s4": s4,
            "s4b": s4b,
        }
        for i in range(NCORES)
    ]


def kernel(x: np.ndarray) -> np.ndarray:
    nc = _get_nc()
    res = run_bass_kernel_spmd(nc, _in_maps(x), list(range(NCORES))).results
    outs = []
    for r in res:
        # y cols: half(P/M)*1024 + j*256 + f2*128 + s ; rows: 4p+j on
        # partition p -> y[4p+j, (half*2+f2)*128+s]
        o = np.asarray(r["y"]).reshape(BPC, 128, 2, 4, 2, 128)
        o = o.transpose(0, 1, 3, 2, 4, 5).reshape(BPC, N, D)
        outs.append(o.astype(np.float32))
    return np.concatenate(outs, axis=0)


# revision 26
# speedup vs baseline: 1.2959x; 1.0020x over previous
import sys

import numpy as np

for p in ("/opt/trn_rl_repo",):
    if p not in sys.path:
        sys.path.insert(0, p)

import concourse.bass as bass  # noqa: E402
import concourse.tile as tile  # noqa: E402
from concourse import bacc, mybir  # noqa: E402
from concourse.bass_utils import run_bass_kernel_spmd  # noqa: E402

B, N, D = 128, 512, 512
NCORES = 8
BPC = B // NCORES  # 16 batch items per core
F32 = mybir.dt.float32
BF16 = mybir.dt.bfloat16


def _hadamard(n: int) -> np.ndarray:
    H = np.array([[1.0]], dtype=np.float32)
    base = np.array([[1.0, 1.0], [1.0, -1.0]], dtype=np.float32)
    while H.shape[0] < n:
        H = np.kron(H, base)
    return H


def _build():
    # Row m = 4q+j lives on partition q, free block j (4 consecutive DRAM
    # rows per partition -> one 512KB DMA per batch with 4KB lines).
    # H512[4q+j, 4p+i] = H128[q,p]*H4[j,i]   (H512 = H128 (x) H4)
    # H512[dc*128+r, f*128+s] = H4[dc,f]*H128[r,s]  (H512 = H4 (x) H128)
    # Both transforms fold one H2 level into PSUM accumulation via paired
    # moving operands [S|S] / [S|-S] (N=256 matmuls); the remaining H2
    # level is one add + one sub on [128,1024] tiles (DVE, 2x rate).
    # Output leaves in (half, j, f', s) column order; host unpermutes.
    nc = bacc.Bacc("TRN2", target_bir_lowering=False, debug=False)
    x_d = nc.dram_tensor("x", [BPC, 128, 4 * D], BF16, kind="ExternalInput").ap()
    s4_d = nc.dram_tensor("s4", [128, 512], BF16, kind="ExternalInput").ap()
    s4b_d = nc.dram_tensor("s4b", [128, 512], BF16, kind="ExternalInput").ap()
    y_d = nc.dram_tensor("y", [BPC, 128, 4 * D], BF16, kind="ExternalOutput").ap()

    with tile.TileContext(nc) as tc:
        with (
            tc.tile_pool(name="const", bufs=1) as const_pool,
            tc.tile_pool(name="xp", bufs=3) as x_pool,
            tc.tile_pool(name="cs", bufs=3) as cs_pool,
            tc.tile_pool(name="csb", bufs=3) as csb_pool,
            tc.tile_pool(name="tp", bufs=3) as t_pool,
            tc.tile_pool(name="yp", bufs=3) as y_pool,
            tc.tile_pool(name="psa", bufs=1, space="PSUM") as psum_a,
            tc.tile_pool(name="psb", bufs=1, space="PSUM") as psum_b,
        ):
            s4_sb = const_pool.tile([128, 512], BF16, tag="s4")
            s4b_sb = const_pool.tile([128, 512], BF16, tag="s4b")

            sp2 = s4_sb[:, 0:256]  # [S | S]
            sn2 = s4_sb[:, 256:512]  # [S | -S]
            sp2b = s4b_sb[:, 0:256]  # [S | S] / 512
            sn2b = s4b_sb[:, 256:512]  # [S | -S] / 512

            state = None  # pending (ttP, ttM, b) from previous batch

            def emit_B(ttP, ttM, bprev, last=False):
                # B: y[4p+j, f*128+s] = sum_dc H4[dc,f] v_dc[4p+j, s]
                # v_dc[n, s] = sum_r t_T[dc*128+r, n] * H128[r,s]/512
                psB01 = psum_b.tile([128, 1024], F32, tag="b01")
                psB23 = psum_b.tile([128, 1024], F32, tag="b23")
                for j in range(4):
                    o = slice(j * 256, j * 256 + 256)
                    src = ttP if j < 2 else ttM
                    jj = (j % 2) * 128
                    t0 = src[:, 0 * 256 + jj : 0 * 256 + jj + 128]
                    t1 = src[:, 1 * 256 + jj : 1 * 256 + jj + 128]
                    t2 = src[:, 2 * 256 + jj : 2 * 256 + jj + 128]
                    t3 = src[:, 3 * 256 + jj : 3 * 256 + jj + 128]
                    nc.tensor.matmul(psB01[:, o], t0, sp2b, start=True, stop=False)
                    nc.tensor.matmul(psB01[:, o], t1, sn2b, start=False, stop=True)
                    nc.tensor.matmul(psB23[:, o], t2, sp2b, start=True, stop=False)
                    nc.tensor.matmul(psB23[:, o], t3, sn2b, start=False, stop=True)
                csB01 = csb_pool.tile([128, 1024], BF16, tag="b01")
                csB23 = csb_pool.tile([128, 1024], BF16, tag="b23")
                nc.scalar.copy(csB01[:], psB01[:])
                nc.vector.tensor_copy(csB23[:], psB23[:])
                yPM = y_pool.tile([128, 2048], BF16, tag="yPM")
                nc.vector.tensor_add(yPM[:, 0:1024], csB01[:], csB23[:])
                if last:
                    # shorten the final-batch tail: ship the P half while
                    # the M half is still being computed
                    nc.sync.dma_start(y_d[bprev][:, 0:1024], yPM[:, 0:1024])
                    nc.vector.tensor_sub(yPM[:, 1024:2048], csB01[:], csB23[:])
                    nc.sync.dma_start(y_d[bprev][:, 1024:2048], yPM[:, 1024:2048])
                else:
                    nc.vector.tensor_sub(yPM[:, 1024:2048], csB01[:], csB23[:])
                    nc.sync.dma_start(y_d[bprev], yPM[:])

            for b in range(BPC):
                # x cols are dt-major: dt*512 + j*128 + dd
                xb = x_pool.tile([128, 4 * D], BF16)
                if b == 0:
                    # consts first (tiny), then x0 in dt-chunks so A(0)
                    # can start right after chunk 0 lands; s4b is only
                    # needed by transform B, so it loads after chunk 0
                    nc.sync.dma_start(s4_sb[:], s4_d[:])
                    nc.sync.dma_start(
                        xb[:, 0:512], x_d[b][:, 0:512]
                    )
                    nc.sync.dma_start(s4b_sb[:], s4b_d[:])
                    for c in range(1, 4):
                        nc.sync.dma_start(
                            xb[:, c * 512 : (c + 1) * 512],
                            x_d[b][:, c * 512 : (c + 1) * 512],
                        )
                    # HAM warmup: the PE clock-gate defaults to 4/8
                    # (1.2 GHz) and needs ~3.4us of sustained activity to
                    # open. Burn the otherwise-idle DMA head with dummy
                    # matmuls on the const tile; batch 0's first real
                    # matmul resets the bank via start=True.
                    warm = psum_a.tile([128, 1024], F32, tag="sd01")
                    for _ in range(16):
                        nc.tensor.matmul(
                            warm[:, 0:256],
                            s4_sb[:, 0:128],
                            s4_sb[:, 0:256],
                            start=True,
                            stop=True,
                        )
                else:
                    nc.sync.dma_start(xb[:], x_d[b])

                # A: u_j[dd, p] = sum_q x[4q+j, dt*128+dd] * H128[q, p]
                # psum free layout dt*256 + (s:0-127 | d:128-255)
                ps_sd01 = psum_a.tile([128, 1024], F32, tag="sd01")
                ps_sd23 = psum_a.tile([128, 1024], F32, tag="sd23")
                for dt in range(4):
                    o = slice(dt * 256, dt * 256 + 256)
                    x0 = xb[:, dt * 512 + 0 : dt * 512 + 128]
                    x1 = xb[:, dt * 512 + 128 : dt * 512 + 256]
                    x2 = xb[:, dt * 512 + 256 : dt * 512 + 384]
                    x3 = xb[:, dt * 512 + 384 : dt * 512 + 512]
                    nc.tensor.matmul(ps_sd01[:, o], x0, sp2, start=True, stop=False)
                    nc.tensor.matmul(ps_sd01[:, o], x1, sn2, start=False, stop=True)
                    nc.tensor.matmul(ps_sd23[:, o], x2, sp2, start=True, stop=False)
                    nc.tensor.matmul(ps_sd23[:, o], x3, sn2, start=False, stop=True)
                cs_sd01 = cs_pool.tile([128, 1024], BF16, tag="sd01")
                cs_sd23 = cs_pool.tile([128, 1024], BF16, tag="sd23")
                nc.scalar.copy(cs_sd01[:, 0:512], ps_sd01[:, 0:512])
                nc.scalar.copy(cs_sd01[:, 512:1024], ps_sd01[:, 512:1024])
                nc.scalar.copy(cs_sd23[:], ps_sd23[:])
                # Remaining H2 level: ttP = [t0|t1], ttM = [t2|t3]
                # layout dt*256 + i'*128 + p
                ttP = t_pool.tile([128, 1024], BF16, tag="P")
                ttM = t_pool.tile([128, 1024], BF16, tag="M")
                nc.vector.tensor_add(ttP[:], cs_sd01[:], cs_sd23[:])
                nc.vector.tensor_sub(ttM[:], cs_sd01[:], cs_sd23[:])
                if state is not None:
                    emit_B(*state)
                state = (ttP, ttM, b)

            emit_B(*state, last=True)

    nc.compile()
    return nc


_NC = None


def _get_nc():
    global _NC
    if _NC is None:
        _NC = _build()
    return _NC


def _in_maps(x: np.ndarray) -> list:
    import ml_dtypes

    bf16 = ml_dtypes.bfloat16
    x = np.asarray(x)
    xb = np.ascontiguousarray(x, dtype=np.float32).astype(bf16)
    H128 = _hadamard(128)
    s4 = np.ascontiguousarray(
        np.concatenate([H128, H128, H128, -H128], axis=1)
    ).astype(bf16)
    s4b = np.ascontiguousarray(s4.astype(np.float32) / np.float32(512.0)).astype(
        bf16
    )
    # device x cols are dt-major: dt*512 + j*128 + dd
    xb = (
        xb.reshape(B, 128, 4, 4, 128)  # (b, q, j, dt, dd)
        .transpose(0, 1, 3, 2, 4)  # (b, q, dt, j, dd)
        .reshape(B, 128, 4 * D)
    )
    return [
        {
            "x": np.ascontiguousarray(xb[i * BPC : (i + 1) * BPC]),
            "s4": s4,
            "s4b": s4b,
        }
        for i in range(NCORES)
    ]


def kernel(x: np.ndarray) -> np.ndarray:
    nc = _get_nc()
    res = run_bass_kernel_spmd(nc, _in_maps(x), list(range(NCORES))).results
    outs = []
    for r in res:
        # y cols: half(P/M)*1024 + j*256 + f2*128 + s ; rows: 4p+j on
        # partition p -> y[4p+j, (half*2+f2)*128+s]
        o = np.asarray(r["y"]).reshape(BPC, 128, 2, 4, 2, 128)
        o = o.transpose(0, 1, 3, 2, 4, 5).reshape(BPC, N, D)
        outs.append(o.astype(np.float32))
    return np.concatenate(outs, axis=0)


# revision 28
# speedup vs baseline: 1.3825x; 1.0668x over previous
import sys

import numpy as np

for p in ("/opt/trn_rl_repo",):
    if p not in sys.path:
        sys.path.insert(0, p)

import concourse.bass as bass  # noqa: E402
import concourse.tile as tile  # noqa: E402
from concourse import bacc, mybir  # noqa: E402
from concourse.bass_utils import run_bass_kernel_spmd  # noqa: E402

B, N, D = 128, 512, 512
NCORES = 8
BPC = B // NCORES  # 16 batch items per core
F32 = mybir.dt.float32
BF16 = mybir.dt.bfloat16


def _hadamard(n: int) -> np.ndarray:
    H = np.array([[1.0]], dtype=np.float32)
    base = np.array([[1.0, 1.0], [1.0, -1.0]], dtype=np.float32)
    while H.shape[0] < n:
        H = np.kron(H, base)
    return H


def _build():
    # Row m = 4q+j lives on partition q, free block j (4 consecutive DRAM
    # rows per partition -> one 512KB DMA per batch with 4KB lines).
    # H512[4q+j, 4p+i] = H128[q,p]*H4[j,i]   (H512 = H128 (x) H4)
    # H512[dc*128+r, f*128+s] = H4[dc,f]*H128[r,s]  (H512 = H4 (x) H128)
    # Both transforms fold one H2 level into PSUM accumulation via paired
    # moving operands [S|S] / [S|-S] (N=256 matmuls); the remaining H2
    # level is one add + one sub on [128,1024] tiles (DVE, 2x rate).
    # Output leaves in (half, j, f', s) column order; host unpermutes.
    nc = bacc.Bacc("TRN2", target_bir_lowering=False, debug=False)
    x_d = nc.dram_tensor("x", [BPC, 128, 4 * D], BF16, kind="ExternalInput").ap()
    s4_d = nc.dram_tensor("s4", [128, 512], BF16, kind="ExternalInput").ap()
    s4b_d = nc.dram_tensor("s4b", [128, 512], BF16, kind="ExternalInput").ap()
    y_d = nc.dram_tensor("y", [BPC, 128, 4 * D], BF16, kind="ExternalOutput").ap()

    with tile.TileContext(nc) as tc:
        with (
            tc.tile_pool(name="const", bufs=1) as const_pool,
            tc.tile_pool(name="xp", bufs=3) as x_pool,
            tc.tile_pool(name="cs", bufs=3) as cs_pool,
            tc.tile_pool(name="csb", bufs=3) as csb_pool,
            tc.tile_pool(name="tp", bufs=3) as t_pool,
            tc.tile_pool(name="yp", bufs=3) as y_pool,
            tc.tile_pool(name="psa", bufs=1, space="PSUM") as psum_a,
            tc.tile_pool(name="psb", bufs=1, space="PSUM") as psum_b,
        ):
            s4_sb = const_pool.tile([128, 512], BF16, tag="s4")
            s4b_sb = const_pool.tile([128, 512], BF16, tag="s4b")

            sp2 = s4_sb[:, 0:256]  # [S | S]
            sn2 = s4_sb[:, 256:512]  # [S | -S]
            sp2b = s4b_sb[:, 0:256]  # [S | S] / 512
            sn2b = s4b_sb[:, 256:512]  # [S | -S] / 512

            state = None  # pending (ttP, ttM, b) from previous batch

            def emit_B(ttP, ttM, bprev, last=False):
                # B: y[4p+j, f*128+s] = sum_dc H4[dc,f] v_dc[4p+j, s]
                # v_dc[n, s] = sum_r t_T[dc*128+r, n] * H128[r,s]/512
                psB01 = psum_b.tile([128, 1024], F32, tag="b01")
                psB23 = psum_b.tile([128, 1024], F32, tag="b23")
                # phase order: complete psB01 first so its drain copy can
                # start while the psB23 matmuls still run
                for j in range(4):
                    o = slice(j * 256, j * 256 + 256)
                    src = ttP if j < 2 else ttM
                    jj = (j % 2) * 128
                    t0 = src[:, 0 * 256 + jj : 0 * 256 + jj + 128]
                    t1 = src[:, 1 * 256 + jj : 1 * 256 + jj + 128]
                    nc.tensor.matmul(psB01[:, o], t0, sp2b, start=True, stop=False)
                    nc.tensor.matmul(psB01[:, o], t1, sn2b, start=False, stop=True)
                for j in range(4):
                    o = slice(j * 256, j * 256 + 256)
                    src = ttP if j < 2 else ttM
                    jj = (j % 2) * 128
                    t2 = src[:, 2 * 256 + jj : 2 * 256 + jj + 128]
                    t3 = src[:, 3 * 256 + jj : 3 * 256 + jj + 128]
                    nc.tensor.matmul(psB23[:, o], t2, sp2b, start=True, stop=False)
                    nc.tensor.matmul(psB23[:, o], t3, sn2b, start=False, stop=True)
                csB01 = csb_pool.tile([128, 1024], BF16, tag="b01")
                csB23 = csb_pool.tile([128, 1024], BF16, tag="b23")
                nc.scalar.copy(csB01[:], psB01[:])
                nc.vector.tensor_copy(csB23[:], psB23[:])
                yPM = y_pool.tile([128, 2048], BF16, tag="yPM")
                nc.vector.tensor_add(yPM[:, 0:1024], csB01[:], csB23[:])
                if last:
                    # shorten the final-batch tail: ship the P half while
                    # the M half is still being computed
                    nc.sync.dma_start(y_d[bprev][:, 0:1024], yPM[:, 0:1024])
                    nc.vector.tensor_sub(yPM[:, 1024:2048], csB01[:], csB23[:])
                    nc.sync.dma_start(y_d[bprev][:, 1024:2048], yPM[:, 1024:2048])
                else:
                    nc.vector.tensor_sub(yPM[:, 1024:2048], csB01[:], csB23[:])
                    nc.sync.dma_start(y_d[bprev], yPM[:])

            for b in range(BPC):
                # x cols are dt-major: dt*512 + j*128 + dd
                xb = x_pool.tile([128, 4 * D], BF16)
                if b == 0:
                    # consts first (tiny), then x0 in dt-chunks so A(0)
                    # can start right after chunk 0 lands; s4b is only
                    # needed by transform B, so it loads after chunk 0
                    nc.sync.dma_start(s4_sb[:], s4_d[:])
                    nc.sync.dma_start(
                        xb[:, 0:512], x_d[b][:, 0:512]
                    )
                    nc.sync.dma_start(s4b_sb[:], s4b_d[:])
                    for c in range(1, 4):
                        nc.sync.dma_start(
                            xb[:, c * 512 : (c + 1) * 512],
                            x_d[b][:, c * 512 : (c + 1) * 512],
                        )
                    # HAM warmup: the PE clock-gate defaults to 4/8
                    # (1.2 GHz) and needs ~3.4us of sustained activity to
                    # open. Burn the otherwise-idle DMA head with dummy
                    # matmuls on the const tile; batch 0's first real
                    # matmul resets the bank via start=True.
                    warm = psum_a.tile([128, 1024], F32, tag="sd01")
                    for _ in range(16):
                        nc.tensor.matmul(
                            warm[:, 0:256],
                            s4_sb[:, 0:128],
                            s4_sb[:, 0:256],
                            start=True,
                            stop=True,
                        )
                else:
                    nc.sync.dma_start(xb[:], x_d[b])

                # A: u_j[dd, p] = sum_q x[4q+j, dt*128+dd] * H128[q, p]
                # psum free layout dt*256 + (s:0-127 | d:128-255)
                ps_sd01 = psum_a.tile([128, 1024], F32, tag="sd01")
                ps_sd23 = psum_a.tile([128, 1024], F32, tag="sd23")
                if b == 0:
                    # dt-chunk order: each dt's MMs gate only on its
                    # x-chunk DMA
                    for dt in range(4):
                        o = slice(dt * 256, dt * 256 + 256)
                        x0 = xb[:, dt * 512 + 0 : dt * 512 + 128]
                        x1 = xb[:, dt * 512 + 128 : dt * 512 + 256]
                        x2 = xb[:, dt * 512 + 256 : dt * 512 + 384]
                        x3 = xb[:, dt * 512 + 384 : dt * 512 + 512]
                        nc.tensor.matmul(
                            ps_sd01[:, o], x0, sp2, start=True, stop=False
                        )
                        nc.tensor.matmul(
                            ps_sd01[:, o], x1, sn2, start=False, stop=True
                        )
                        nc.tensor.matmul(
                            ps_sd23[:, o], x2, sp2, start=True, stop=False
                        )
                        nc.tensor.matmul(
                            ps_sd23[:, o], x3, sn2, start=False, stop=True
                        )
                else:
                    # phase order: finish ps_sd01 first so its drain copy
                    # overlaps the ps_sd23 matmuls
                    for dt in range(4):
                        o = slice(dt * 256, dt * 256 + 256)
                        x0 = xb[:, dt * 512 + 0 : dt * 512 + 128]
                        x1 = xb[:, dt * 512 + 128 : dt * 512 + 256]
                        nc.tensor.matmul(
                            ps_sd01[:, o], x0, sp2, start=True, stop=False
                        )
                        nc.tensor.matmul(
                            ps_sd01[:, o], x1, sn2, start=False, stop=True
                        )
                    for dt in range(4):
                        o = slice(dt * 256, dt * 256 + 256)
                        x2 = xb[:, dt * 512 + 256 : dt * 512 + 384]
                        x3 = xb[:, dt * 512 + 384 : dt * 512 + 512]
                        nc.tensor.matmul(
                            ps_sd23[:, o], x2, sp2, start=True, stop=False
                        )
                        nc.tensor.matmul(
                            ps_sd23[:, o], x3, sn2, start=False, stop=True
                        )
                cs_sd01 = cs_pool.tile([128, 1024], BF16, tag="sd01")
                cs_sd23 = cs_pool.tile([128, 1024], BF16, tag="sd23")
                nc.scalar.copy(cs_sd01[:], ps_sd01[:])
                nc.scalar.copy(cs_sd23[:], ps_sd23[:])
                # Remaining H2 level: ttP = [t0|t1], ttM = [t2|t3]
                # layout dt*256 + i'*128 + p
                ttP = t_pool.tile([128, 1024], BF16, tag="P")
                ttM = t_pool.tile([128, 1024], BF16, tag="M")
                nc.vector.tensor_add(ttP[:], cs_sd01[:], cs_sd23[:])
                nc.vector.tensor_sub(ttM[:], cs_sd01[:], cs_sd23[:])
                if state is not None:
                    emit_B(*state)
                state = (ttP, ttM, b)

            emit_B(*state, last=True)

    nc.compile()
    return nc


_NC = None


def _get_nc():
    global _NC
    if _NC is None:
        _NC = _build()
    return _NC


def _in_maps(x: np.ndarray) -> list:
    import ml_dtypes

    bf16 = ml_dtypes.bfloat16
    x = np.asarray(x)
    xb = np.ascontiguousarray(x, dtype=np.float32).astype(bf16)
    H128 = _hadamard(128)
    s4 = np.ascontiguousarray(
        np.concatenate([H128, H128, H128, -H128], axis=1)
    ).astype(bf16)
    s4b = np.ascontiguousarray(s4.astype(np.float32) / np.float32(512.0)).astype(
        bf16
    )
    # device x cols are dt-major: dt*512 + j*128 + dd
    xb = (
        xb.reshape(B, 128, 4, 4, 128)  # (b, q, j, dt, dd)
        .transpose(0, 1, 3, 2, 4)  # (b, q, dt, j, dd)
        .reshape(B, 128, 4 * D)
    )
    return [
        {
            "x": np.ascontiguousarray(xb[i * BPC : (i + 1) * BPC]),
            "s4": s4,
            "s4b": s4b,
        }
        for i in range(NCORES)
    ]


def kernel(x: np.ndarray) -> np.ndarray:
    nc = _get_nc()
    res = run_bass_kernel_spmd(nc, _in_maps(x), list(range(NCORES))).results
    outs = []
    for r in res:
        # y cols: half(P/M)*1024 + j*256 + f2*128 + s ; rows: 4p+j on
        # partition p -> y[4p+j, (half*2+f2)*128+s]
        o = np.asarray(r["y"]).reshape(BPC, 128, 2, 4, 2, 128)
        o = o.transpose(0, 1, 3, 2, 4, 5).reshape(BPC, N, D)
        outs.append(o.astype(np.float32))
    return np.concatenate(outs, axis=0)
